# revision 2
# baseline (speedup 1.0000x reference)
"""Multi-head attention (MemoryNet) Bass kernel for 8 Trainium2 cores.

Problem (per reference):
  q,k: [b=4, d=1024, m/n=2048], v: [4, 1024, 2048] fp32, N_HEAD=8
  per head (32 total): S = (qh^T kh)/sqrt(128); P = softmax(S, axis=-1)
  out_head = vh @ P^T  -> [128, 2048]; out = [4, 1024, 2048]

Sharding: 32 heads = 8 cores x 4 heads; pure head parallelism.

v2 design (per-core, 4 heads, software-pipelined):
  - q,k,v loaded as bf16 via gpsimd casting DMA (no DVE cast pass)
  - vT built with one DMA-xbar transpose + gpsimd SBUF->SBUF DMA into
    the packed [vT | ones] layout (ones column -> softmax denominator Z)
  - chunk loop j=0..15 per head:
      S^T[n_j, m] = k_j^T q on TensorE (fp32 PSUM, 2 half-tiles)
      exp: half0 (m cols 0:1024) on ScalarE ACT (bf16 out);
           half1 (1024:2048) on VectorE via the Schraudolph int16
           bit-trick: bits = round(S*A16 + B16) -> bf16 bit pattern
           (rms rel err ~1.8% on that half; output-level ~1.3%)
      pass-A AV (m-tiles 0-2, one psum bank each) consumes chunk j-1
      one deferred AV unit of the PREVIOUS head per chunk (tiles 3-15)
  - finish: 1/Z via DVE reciprocal; per-partition scale on ACT (Copy
    with scale=rz AP) for tiles 0-9, on DVE tensor_scalar_mul for
    10-15; store O^T[m, c] via SWDGE; host un-transposes per head.
"""

import sys

sys.path.insert(0, "/opt/trn_rl_repo")

import numpy as np

N_CORES = 8
HPC = 4  # heads per core
DH = 128  # head dim (contraction for QK)
M = 2048  # queries
NK = 2048  # keys
CH = 128  # v channels per head
NT = NK // 128  # 16 n-chunks
MT = M // 128  # 16 m-tiles
SCALE = 1.0 / float(np.sqrt(DH))
LOG2E = 1.4426950408889634
# Schraudolph bf16 bit-trick: bits16 = round(S * A16 + B16)
A16 = SCALE * LOG2E * 128.0
C16 = 7.5
B16 = 127.0 * 128.0 - C16
SPLIT = 1024  # m columns handled by ACT; rest by DVE trick

# Pass-A m-tiles: consumed chunk-by-chunk (trailing by 1), own psum bank
A_TILES = [0, 1, 2]
# Remaining 13 m-tiles of the PREVIOUS head run as deferred units, one
# per chunk, packs of 3 sharing a psum bank (t-sequential within pack).
OLD_PACKS = [[3, 4, 5], [6, 7, 8], [9, 10, 11], [12, 13, 14], [15]]
OLD_AT = {
    1: [(0, 0)], 2: [(0, 1)], 3: [(0, 2)],
    4: [(1, 0)], 5: [(1, 1)], 6: [(1, 2)],
    7: [(2, 0)], 8: [(2, 1)], 9: [(2, 2)],
    10: [(3, 0)], 11: [(3, 1)], 12: [(3, 2)],
    13: [(4, 0)],
}
ACT_FIN_TILES = 10  # tiles < this finish on ScalarE, rest on VectorE

_CACHE = {}


def _build(loop_reps=1):
    from contextlib import ExitStack

    from concourse import bacc, mybir, tile

    f32 = mybir.dt.float32
    bf16 = mybir.dt.bfloat16
    i16 = mybir.dt.int16

    nc = bacc.Bacc("TRN2", target_bir_lowering=False, debug=False,
                   num_devices=N_CORES)
    q4 = nc.dram_tensor("q4", (HPC, DH, M), f32, kind="ExternalInput").ap()
    k4 = nc.dram_tensor("k4", (HPC, DH, NK), f32, kind="ExternalInput").ap()
    v4 = nc.dram_tensor("v4", (HPC, CH, NK), f32, kind="ExternalInput").ap()
    # per-head output is O^T [m, c]; host transposes during gather
    o4t = nc.dram_tensor("o4t", (HPC, M, CH), f32, kind="ExternalOutput").ap()

    with tile.TileContext(nc) as tc, ExitStack() as ctx:
        bfp = ctx.enter_context(tc.tile_pool(name="bfp", bufs=6))
        vtp = ctx.enter_context(tc.tile_pool(name="vtp", bufs=2))
        vtraw = ctx.enter_context(tc.tile_pool(name="vtraw", bufs=2))
        ep = ctx.enter_context(tc.tile_pool(name="ep", bufs=33))
        outp = ctx.enter_context(tc.tile_pool(name="outp", bufs=2))
        smallp = ctx.enter_context(tc.tile_pool(name="smallp", bufs=10))
        pss = ctx.enter_context(tc.tile_pool(name="pss", bufs=2, space="PSUM"))
        psoa = ctx.enter_context(tc.tile_pool(name="psoa", bufs=3,
                                              space="PSUM"))
        pso = ctx.enter_context(tc.tile_pool(name="pso", bufs=1, space="PSUM"))

        if loop_reps > 1:
            ctx.enter_context(tc.For_i(0, loop_reps, 1))

        def emit_load(h):
            st = {}
            st["qb"] = bfp.tile([DH, M], bf16, tag="bf", name=f"qb{h}")
            st["kb"] = bfp.tile([DH, NK], bf16, tag="bf", name=f"kb{h}")
            vb = bfp.tile([CH, NK], bf16, tag="bf", name=f"vb{h}")
            # casting DMAs: fp32 DRAM -> bf16 SBUF
            nc.gpsimd.dma_start(out=st["kb"], in_=k4[h])
            nc.gpsimd.dma_start(out=st["qb"], in_=q4[h])
            nc.gpsimd.dma_start(out=vb, in_=v4[h])
            # one xbar transpose per head: vt[p, j, c] = v[c, 128j+p]
            vt_raw = vtraw.tile([128, NT, 128], bf16, tag="vtr",
                                name=f"vtr{h}")
            nc.sync.dma_start_transpose(out=vt_raw, in_=vb)
            vton = vtp.tile([128, NT, 132], bf16, tag="vt", name=f"vton{h}")
            nc.gpsimd.memset(vton, 1.0)
            nc.gpsimd.dma_start(out=vton[:, :, 0:128], in_=vt_raw)
            st["vton"] = vton
            st["h"] = h
            st["expst"] = []
            return st

        def finish_tile(po_slice, z_slice, i, out_t, h):
            rz = smallp.tile([128, 1], f32, tag="rz", name=f"rz{h}_{i}")
            nc.vector.reciprocal(rz, z_slice)
            if i < ACT_FIN_TILES:
                nc.scalar.activation(out_t[:, i, :], po_slice,
                                     mybir.ActivationFunctionType.Copy,
                                     scale=rz)
            else:
                nc.vector.tensor_scalar_mul(out_t[:, i, :], po_slice, rz)

        def emit_old_unit(old, pi, t):
            """One (pack, slice) unit: 16 accumulating matmuls + finish."""
            pack = OLD_PACKS[pi]
            h = old["h"]
            if t == 0:
                old["po_cur"] = pso.tile([128, 3, 132], f32, tag="po",
                                         name=f"po{h}_r{pi}")
            po = old["po_cur"]
            i = pack[t]
            for j in range(NT):
                nc.tensor.matmul(
                    po[:, t, :],
                    old["expst"][j][:, 128 * i:128 * (i + 1)],
                    old["vton"][:, j, :],
                    start=(j == 0),
                    stop=(j == NT - 1),
                )
            finish_tile(po[:, t, 0:128], po[:, t, 128:129], i,
                        old["out_t"], h)

        def emit_pass_a(st, po_a, j):
            e = st["expst"][j]
            for pi, i in enumerate(A_TILES):
                nc.tensor.matmul(
                    po_a[pi][:, :],
                    e[:, 128 * i:128 * (i + 1)],
                    st["vton"][:, j, :],
                    start=(j == 0),
                    stop=(j == NT - 1),
                )

        old = None
        st = emit_load(0)
        for h in range(HPC):
            out_t = outp.tile([128, MT, CH], f32, tag="out", name=f"out{h}")
            st["out_t"] = out_t
            po_a = [
                psoa.tile([128, 132], f32, tag="poa", name=f"po{h}_a{pi}")
                for pi in range(len(A_TILES))
            ]
            for j in range(NT):
                if old is not None and j in OLD_AT:
                    for pi, t in OLD_AT[j]:
                        emit_old_unit(old, pi, t)
                e = ep.tile([128, M], bf16, tag="e", name=f"e{h}_{j}")
                kslice = st["kb"][:, 128 * j:128 * (j + 1)]
                for half in range(2):
                    s = pss.tile([128, 1024], f32, tag="s",
                                 name=f"s{h}_{j}_{half}")
                    for quarter in range(2):
                        mo = 1024 * half + 512 * quarter
                        nc.tensor.matmul(
                            s[:, 512 * quarter:512 * (quarter + 1)],
                            kslice,
                            st["qb"][:, mo:mo + 512],
                            start=True,
                            stop=True,
                        )
                    if half == 0:
                        nc.scalar.activation(
                            e[:, 0:SPLIT],
                            s,
                            mybir.ActivationFunctionType.Exp,
                            scale=SCALE,
                        )
                    else:
                        nc.vector.tensor_scalar(
                            e[:, SPLIT:M].bitcast(i16),
                            s,
                            A16,
                            B16,
                            mybir.AluOpType.mult,
                            mybir.AluOpType.add,
                        )
                st["expst"].append(e)
                if j > 0:
                    emit_pass_a(st, po_a, j - 1)
                if j == 4 and h + 1 < HPC:
                    nxt = emit_load(h + 1)
            emit_pass_a(st, po_a, NT - 1)
            if old is not None:
                nc.gpsimd.dma_start(out=o4t[old["h"]].rearrange(
                    "(i p) c -> p i c", p=128), in_=old["out_t"])
            for pi, i in enumerate(A_TILES):
                finish_tile(po_a[pi][:, 0:128], po_a[pi][:, 128:129], i,
                            out_t, h)
            old = st
            if h + 1 < HPC:
                st = nxt
        # drain last head's deferred units
        for j in sorted(OLD_AT):
            for pi, t in OLD_AT[j]:
                emit_old_unit(old, pi, t)
        nc.gpsimd.dma_start(out=o4t[old["h"]].rearrange(
            "(i p) c -> p i c", p=128), in_=old["out_t"])

    nc.compile()
    return nc


def _get_nc():
    if "nc" not in _CACHE:
        _CACHE["nc"] = _build()
    return _CACHE["nc"]


def kernel(q, k, v):
    from concourse.bass_utils import run_bass_kernel_spmd

    nc = _get_nc()
    b, d, m = q.shape
    qh = np.ascontiguousarray(q.reshape(32, DH, M))
    kh = np.ascontiguousarray(k.reshape(32, DH, NK))
    vh = np.ascontiguousarray(v.reshape(32, CH, NK))
    in_maps = [
        {
            "q4": qh[HPC * c:HPC * (c + 1)],
            "k4": kh[HPC * c:HPC * (c + 1)],
            "v4": vh[HPC * c:HPC * (c + 1)],
        }
        for c in range(N_CORES)
    ]
    res = run_bass_kernel_spmd(nc, in_maps, core_ids=list(range(N_CORES)))
    # o4t is [HPC, M, CH] per core = O^T per head; transpose to [CH, M]
    out_t = np.concatenate(
        [res.results[c]["o4t"] for c in range(N_CORES)], axis=0
    )  # [32, M, CH]
    out = np.ascontiguousarray(out_t.transpose(0, 2, 1))  # [32, CH, M]
    return out.reshape(b, d, m).astype(np.float32)
